# revision 11
# baseline (speedup 1.0000x reference)
"""Trainium2 Bass kernel for nn_Decoder (latent-grid decoder MLP).

Contract: kernel(**inputs) takes the FULL unsharded inputs (as produced by
setup_inputs()) and returns the FULL [65536, 4] float32 output. Internally the
65536 points are sharded across 8 NeuronCores (pure data parallel); the small
weights are replicated.

Algorithm (mathematically equivalent to the reference):
  - G=2 trilinear interp of a per-sample 2x2x2 grid always lands in cell
    (0,0,0), so lat_i = sum_m w_m(xyz) * (lat @ A_m).
  - Expressed in the MONOMIAL basis: lat_i @ W0_top = lat@D_0 + sum_{S}
    mono_S(f) * (lat @ D_S) where D_S are alternating sums of the corner
    matrices folded with W0 (host-side).  u = [lat, fx*lat, ..., fxfyfz*lat,
    sin, cos] (2304 dims), h0 = u @ M0.
  - LayerNorm mean-subtraction and gamma fold into the weights; the per-sample
    rstd is deferred via LN's positive scale invariance.  Only the last two
    layers' sum-of-squares are needed: c7^2 = S7 + eps*S6, out = y/c7.
    S_j is accumulated by the PE with a per-partition-scaled Square on ACT
    (scale 1/(sqrt(512)*g)) and an all-ones [128,4] stationary.
  - Per-block schedule is software-pipelined: the whole preamble of block b+1
    (input transposes, trilinear monomials, Fourier range reduction + sin/cos,
    broadcast matmuls and u-chunk products) is emitted interleaved with block
    b's MLP layers, so the PE sees a dense stream of 512-row fp32r matmuls.
Activations live in [feature, sample] layout; matmuls run as fp32r (full PE
rate at N=512).
"""

import os
import numpy as np

N_CORES = 8
N_TOTAL = 65536
S_CORE = N_TOTAL // N_CORES          # 8192 samples per core
BLK = 512                            # samples per block
N_BLOCKS = S_CORE // BLK             # 16
EPS = 1e-5
N_LAYERS = 8                         # LN+relu layers (layer0 + 7 hidden)


def _precompute(inputs):
    """Host-side weight folding. Returns dict of constant arrays (fp32)."""
    convT_w = np.asarray(inputs["convT_w"], np.float32)
    W0 = np.asarray(inputs["W0"], np.float32)
    Wh = np.asarray(inputs["Wh"], np.float32)
    ln_g = np.asarray(inputs["ln_g"], np.float32)
    gauss = np.asarray(inputs["gauss"], np.float32)
    W_out = np.asarray(inputs["W_out"], np.float32)

    # corner-folded first-layer weights: B[d] = A_d @ W0_top, [2,2,2,256,512]
    A = convT_w.transpose(2, 3, 4, 0, 1).reshape(8, 256, 512)
    B = (A @ W0[:512]).reshape(2, 2, 2, 256, 512)
    # monomial basis D_S = sum_{d subset S} (-1)^{|S|-|d|} B_d
    D = np.empty((8, 256, 512), np.float32)
    D[0] = B[0, 0, 0]
    D[1] = B[1, 0, 0] - B[0, 0, 0]                                   # fx
    D[2] = B[0, 1, 0] - B[0, 0, 0]                                   # fy
    D[3] = B[0, 0, 1] - B[0, 0, 0]                                   # fz
    D[4] = B[1, 1, 0] - B[1, 0, 0] - B[0, 1, 0] + B[0, 0, 0]         # fx fy
    D[5] = B[1, 0, 1] - B[1, 0, 0] - B[0, 0, 1] + B[0, 0, 0]         # fx fz
    D[6] = B[0, 1, 1] - B[0, 1, 0] - B[0, 0, 1] + B[0, 0, 0]         # fy fz
    D[7] = (B[1, 1, 1] - B[1, 1, 0] - B[1, 0, 1] - B[0, 1, 1]
            + B[1, 0, 0] + B[0, 1, 0] + B[0, 0, 1] - B[0, 0, 0])     # fx fy fz
    M0 = np.concatenate([D.reshape(2048, 512), W0[512:640], W0[640:768]], axis=0)

    def center_scale(W, g):
        Wc = W - W.mean(axis=1, keepdims=True)
        return np.ascontiguousarray(Wc * g[None, :], np.float32)

    W_eff = [center_scale(M0, ln_g[0])] + [
        center_scale(Wh[l], ln_g[l + 1]) for l in range(7)
    ]

    # pack each layer's weights as [128, n_kchunks, 512]
    def pack(W):
        K = W.shape[0]
        kc = K // 128
        return W.reshape(kc, 128, 512).transpose(1, 0, 2).reshape(128, kc * 512)

    w0p = np.ascontiguousarray(pack(W_eff[0]))                       # [128, 18*512]
    whp = np.ascontiguousarray(
        np.concatenate([pack(W) for W in W_eff[1:]], axis=1))        # [128, 28*512]

    # per-partition ACT Square scales: col (j-6)*4+mc -> 1/(sqrt(512)*|g_j|)
    sqs = np.empty((128, 8), np.float32)
    for j in (6, 7):
        g = np.abs(ln_g[j]).astype(np.float32)
        for mc in range(4):
            sqs[:, (j - 6) * 4 + mc] = 1.0 / (np.sqrt(512.0) * g[mc * 128:(mc + 1) * 128])

    # gauss stationaries: [4, 2*128]; col block 0 = gauss.T (row 3 zero),
    # col block 1 = gauss.T with row 3 = 0.25 (cos phase shift, revolutions)
    gq = np.zeros((4, 256), np.float32)
    gq[0:3, 0:128] = gauss.T
    gq[0:3, 128:256] = gauss.T
    gq[3, 128:256] = 0.25

    # monomial broadcast selector, rows 32:39 (matmul tile_position row=32)
    sel7 = np.zeros((39, 7 * 128), np.float32)
    sel7[32:39] = np.kron(np.eye(7, dtype=np.float32), np.ones((1, 128), np.float32))

    return {
        "w0p": w0p,
        "whp": whp,
        "sqs": sqs,
        "gq": gq,
        "sel7": np.ascontiguousarray(sel7),
        "ident": np.eye(128, dtype=np.float32),
        "ones44": np.ones((128, 4), np.float32),
        "woutp": np.ascontiguousarray(
            W_out.reshape(4, 128, 4).transpose(1, 0, 2).reshape(128, 16)),
    }


def _general_case_needed(inputs):
    z = lambda a: bool(np.all(np.asarray(a) == 0))
    return not (
        z(inputs["convT_b"]) and z(inputs["b0"]) and z(inputs["bh"])
        and z(inputs["ln_b"]) and z(inputs["b_out"])
        and bool(np.all(np.abs(np.asarray(inputs["ln_g"])) > 1e-3))
    )


def _numpy_fallback(inputs):
    """Reference in numpy (slow; only for inputs outside the fast path)."""
    inp = np.asarray(inputs["input"], np.float32)
    convT_w = np.asarray(inputs["convT_w"], np.float32)
    convT_b = np.asarray(inputs["convT_b"], np.float32)
    gauss = np.asarray(inputs["gauss"], np.float32)
    W0 = np.asarray(inputs["W0"], np.float32)
    b0 = np.asarray(inputs["b0"], np.float32)
    Wh = np.asarray(inputs["Wh"], np.float32)
    bh = np.asarray(inputs["bh"], np.float32)
    ln_g = np.asarray(inputs["ln_g"], np.float32)
    ln_b = np.asarray(inputs["ln_b"], np.float32)
    W_out = np.asarray(inputs["W_out"], np.float32)
    b_out = np.asarray(inputs["b_out"], np.float32)
    xyz = inp[:, -3:]
    lat = inp[:, :-3]
    f = (xyz + 1.0) * 0.5
    frac = f - np.clip(f.astype(np.int32), 0, 0)
    A = convT_w.transpose(2, 3, 4, 0, 1)
    lat_i = np.zeros((inp.shape[0], 512), np.float32)
    wx = [1 - frac[:, 0], frac[:, 0]]
    wy = [1 - frac[:, 1], frac[:, 1]]
    wz = [1 - frac[:, 2], frac[:, 2]]
    for di in (0, 1):
        for dj in (0, 1):
            for dk in (0, 1):
                w = (wx[di] * wy[dj] * wz[dk]).astype(np.float32)
                lat_i += (lat @ A[di, dj, dk]) * w[:, None]
    lat_i += convT_b[None, :]
    ang = 2 * np.pi * (xyz @ gauss.T)
    x = np.concatenate([lat_i, np.sin(ang), np.cos(ang)], axis=1)

    def ln(t, g, b):
        mu = t.mean(-1, keepdims=True)
        var = ((t - mu) ** 2).mean(-1, keepdims=True)
        return (t - mu) / np.sqrt(var + EPS) * g + b

    x = np.maximum(ln(x @ W0 + b0, ln_g[0], ln_b[0]), 0)
    for l in range(7):
        x = np.maximum(ln(x @ Wh[l] + bh[l], ln_g[l + 1], ln_b[l + 1]), 0)
    y = x @ W_out + b_out
    return np.concatenate([np.tanh(y[:, :1]), y[:, 1:] * 255.0], axis=1).astype(np.float32)


_NC_CACHE = {}


def _build_bass(s_core=S_CORE):
    """Build the per-core Bass module (SPMD; same program on all 8 cores)."""
    import concourse.bass as bass
    import concourse.bacc as bacc
    import concourse.tile as tile
    from concourse import mybir

    FP32 = mybir.dt.float32
    FP32R = mybir.dt.float32r
    INT32 = mybir.dt.int32
    AF = mybir.ActivationFunctionType
    ALU = mybir.AluOpType
    TWO_PI = float(2.0 * np.pi)
    MAGIC = 12582912.0            # 1.5 * 2^23: fp32 add/sub rounds to integer
    n_blocks = s_core // BLK

    nc = bacc.Bacc("TRN2", target_bir_lowering=False, debug=False)

    inp_d = nc.dram_tensor("inp", [s_core, 259], FP32R, kind="ExternalInput").ap()
    w0p_d = nc.dram_tensor("w0p", [128, 18 * 512], FP32R, kind="ExternalInput").ap()
    whp_d = nc.dram_tensor("whp", [128, 28 * 512], FP32R, kind="ExternalInput").ap()
    sqs_d = nc.dram_tensor("sqs", [128, 8], FP32, kind="ExternalInput").ap()
    gq_d = nc.dram_tensor("gq", [4, 256], FP32R, kind="ExternalInput").ap()
    sel7_d = nc.dram_tensor("sel7", [39, 7 * 128], FP32R, kind="ExternalInput").ap()
    ident_d = nc.dram_tensor("ident", [128, 128], FP32R, kind="ExternalInput").ap()
    ones44_d = nc.dram_tensor("ones44", [128, 4], FP32R, kind="ExternalInput").ap()
    woutp_d = nc.dram_tensor("woutp", [128, 16], FP32R, kind="ExternalInput").ap()
    outT_d = nc.dram_tensor("outT", [4, s_core], FP32, kind="ExternalOutput").ap()

    def R(ap):
        return ap.bitcast(FP32R)

    with tile.TileContext(nc) as tc:
        with (
            tc.tile_pool(name="const", bufs=1) as constp,
            tc.tile_pool(name="weights", bufs=1) as weightp,
            tc.tile_pool(name="inblk", bufs=2) as inp_pool,
            tc.tile_pool(name="pre", bufs=2) as prep,
            tc.tile_pool(name="acts", bufs=2) as actp,
            tc.tile_pool(name="scratch", bufs=2) as scr,
            tc.tile_pool(name="ps_t", bufs=1, space="PSUM") as ps_t,
            tc.tile_pool(name="ps_share", bufs=1, space="PSUM") as ps_share,
            tc.tile_pool(name="ps_pre", bufs=1, space="PSUM") as ps_pre,
        ):
            inp_r = inp_d.rearrange("(b sc p) f -> b p sc f", sc=4, p=128)

            def make_pre(b):
                """Preamble for block b: returns (tiles dict, list of closures).

                Issues the input DMA immediately; everything else is deferred
                into steps that the caller pumps between M-phase matmul groups
                of block b-1.  Produces latT, movers (18 l0 moving chunks) in
                SBUF, all in [feature, sample] layout.
                """
                t = {}
                inb = inp_pool.tile([128, 4, 259], FP32R, tag="inb", name="inb")
                nc.sync.dma_start(out=inb, in_=inp_r[b])
                t["latT"] = prep.tile([128, 2, BLK], FP32R, tag="latT", name="latT")
                wxz = prep.tile([128, 4, 39], FP32R, tag="wxz", name="wxz")
                xyzq = prep.tile([4, BLK], FP32R, tag="xyzq", name="xyzq")
                w7T = prep.tile([39, BLK], FP32R, tag="w7T", name="w7T")
                ffs = prep.tile([128, BLK], FP32R, tag="ffs", name="ffs")
                ffc = prep.tile([128, BLK], FP32R, tag="ffc", name="ffc")
                uchs = [scr.tile([128, BLK], FP32R, tag="uch", bufs=14, name=f"uch{m}")
                        for m in range(14)]
                # l0 moving chunks in order matching M0 rows
                t["movers"] = [t["latT"][:, 0, :], t["latT"][:, 1, :]] + \
                    [uchs[i] for i in range(14)] + [ffs, ffc]
                steps = []

                def s_wxz():
                    # f = (xyz+1)/2 into monomial cols 32:35; products 35:39;
                    # raw xyz into 0:3; ones into col 3
                    nc.vector.tensor_scalar(
                        out=wxz[:, :, 32:35], in0=inb[:, :, 256:259],
                        scalar1=0.5, scalar2=0.5, op0=ALU.mult, op1=ALU.add)
                    nc.vector.tensor_tensor(
                        out=wxz[:, :, 35:36], in0=wxz[:, :, 32:33],
                        in1=wxz[:, :, 33:34], op=ALU.mult)              # fx fy
                    nc.vector.tensor_tensor(
                        out=wxz[:, :, 36:37], in0=wxz[:, :, 32:33],
                        in1=wxz[:, :, 34:35], op=ALU.mult)              # fx fz
                    nc.vector.tensor_tensor(
                        out=wxz[:, :, 37:38], in0=wxz[:, :, 33:34],
                        in1=wxz[:, :, 34:35], op=ALU.mult)              # fy fz
                    nc.vector.tensor_tensor(
                        out=wxz[:, :, 38:39], in0=wxz[:, :, 35:36],
                        in1=wxz[:, :, 34:35], op=ALU.mult)              # fx fy fz
                    nc.vector.tensor_copy(out=wxz[:, :, 0:3], in_=inb[:, :, 256:259])
                    nc.vector.tensor_scalar(
                        out=wxz[:, :, 3:4], in0=inb[:, :, 0:1],
                        scalar1=0.0, scalar2=1.0, op0=ALU.mult, op1=ALU.add)
                steps.append(s_wxz)

                # per-sc: 2 lat transposes + 1 combined xyz/monomial transpose
                def mk_lat_tp(sc, fc):
                    def s():
                        tp = ps_pre.tile([128, 128], FP32R, tag="tp", bufs=1, name="tp")
                        nc.tensor.transpose(
                            tp, inb[:, sc, fc * 128:(fc + 1) * 128], ident_sb)
                        nc.vector.tensor_copy(
                            t["latT"][:, fc, sc * 128:(sc + 1) * 128], tp)
                    return s

                def mk_wxz_tp(sc):
                    def s():
                        tp = ps_pre.tile([39, 128], FP32R, tag="tp", bufs=1, name="tpw")
                        nc.tensor.transpose(tp, wxz[:, sc, :], ident_sb)
                        nc.vector.tensor_copy(
                            xyzq[:, sc * 128:(sc + 1) * 128], tp[0:4, :])
                        nc.vector.tensor_copy(
                            w7T[32:39, sc * 128:(sc + 1) * 128], tp[32:39, :])
                    return s

                for sc in range(4):
                    steps.append(mk_lat_tp(sc, 0))
                    steps.append(mk_lat_tp(sc, 1))
                    steps.append(mk_wxz_tp(sc))

                # fourier: ang matmul + range reduce + sin (and cos phase)
                def mk_ang(col, zname, fout):
                    def s():
                        k = 3 if col == 0 else 4
                        angp = ps_pre.tile([128, BLK], FP32, tag="ang", bufs=1, name="angp")
                        nc.tensor.matmul(
                            angp, gq_sb[0:k, col, :], xyzq[0:k, :],
                            start=True, stop=True)
                        zr = scr.tile([128, BLK], FP32, tag="zr", bufs=1, name=zname + "r")
                        nc.vector.tensor_scalar(
                            out=zr, in0=angp, scalar1=MAGIC, scalar2=MAGIC,
                            op0=ALU.add, op1=ALU.subtract)
                        zz = scr.tile([128, BLK], FP32, tag="zz", bufs=1, name=zname)
                        nc.vector.tensor_sub(zz, angp, zr)
                        nc.scalar.activation(out=fout, in_=zz, func=AF.Sin, scale=TWO_PI)
                    return s

                steps.append(mk_ang(0, "zs", ffs))
                steps.append(mk_ang(1, "zc", ffc))

                # broadcast + u-chunk products (consumed by l0 of block b)
                def mk_bc(m):
                    def s():
                        bcp = ps_share.tile([128, BLK], FP32, tag="bc", bufs=2, name="bcp")
                        nc.tensor.matmul(
                            bcp, sel7_sb[32:39, m, :], w7T[32:39, :],
                            start=True, stop=True, tile_position=(32, 0))
                        nc.vector.tensor_tensor(
                            out=uchs[2 * m], in0=t["latT"][:, 0, :], in1=bcp,
                            op=ALU.mult)
                        nc.vector.tensor_tensor(
                            out=uchs[2 * m + 1], in0=t["latT"][:, 1, :], in1=bcp,
                            op=ALU.mult)
                    return s

                for m in range(7):
                    steps.append(mk_bc(m))
                return t, steps

            pre_next, steps0 = make_pre(0)

            # ---- constants / weights (loaded once, resident) ----
            # (first block's input DMA is issued below, before these bulk
            # weight transfers, so the prologue transposes can start early)
            w0_sb = weightp.tile([128, 18, 512], FP32R)
            w0r = w0p_d.rearrange("p (c f) -> p c f", c=18)
            for ch in range(3):
                nc.sync.dma_start(
                    out=w0_sb[:, ch * 6:(ch + 1) * 6, :], in_=w0r[:, ch * 6:(ch + 1) * 6, :])
            wh_sb = weightp.tile([128, 28, 512], FP32R)
            whr = whp_d.rearrange("p (c f) -> p c f", c=28)
            for ch in range(4):
                nc.sync.dma_start(
                    out=wh_sb[:, ch * 7:(ch + 1) * 7, :], in_=whr[:, ch * 7:(ch + 1) * 7, :])
            sqs_sb = constp.tile([128, 8], FP32)
            nc.sync.dma_start(out=sqs_sb, in_=sqs_d)
            gq_sb = constp.tile([4, 2, 128], FP32R)
            nc.sync.dma_start(out=gq_sb, in_=gq_d.rearrange("p (c f) -> p c f", c=2))
            sel7_sb = constp.tile([39, 7, 128], FP32R)
            nc.sync.dma_start(out=sel7_sb, in_=sel7_d.rearrange("p (m f) -> p m f", m=7))
            ident_sb = constp.tile([128, 128], FP32R)
            nc.sync.dma_start(out=ident_sb, in_=ident_d)
            ones44_sb = constp.tile([128, 4], FP32R)
            nc.sync.dma_start(out=ones44_sb, in_=ones44_d)
            wout_sb = weightp.tile([128, 4, 4], FP32R)
            nc.sync.dma_start(out=wout_sb, in_=woutp_d.rearrange("p (c f) -> p c f", c=4))

            def emit_M(b, pre):
                """MLP phase for block b; returns a finalize closure."""
                movers = pre["movers"]

                def finalize_stub():
                    pass

                # ---- layer 0: mc-major, 18 accumulating matmuls per chunk ----
                x_cur = actp.tile([128, 4, BLK], FP32R, tag="xn", name="x0")
                for mc in range(4):
                    ps = ps_t.tile([128, BLK], FP32, tag="pt", bufs=3, name="ps0")
                    for kc in range(18):
                        nc.tensor.matmul(
                            ps, w0_sb[:, kc, mc * 128:(mc + 1) * 128], movers[kc],
                            start=(kc == 0), stop=(kc == 17))
                    nc.scalar.activation(out=x_cur[:, mc, :], in_=ps, func=AF.Relu)
                    yield
                # ---- hidden layers ----
                g67 = [None, None]
                for j in range(1, N_LAYERS):
                    x_next = actp.tile([128, 4, BLK], FP32R, tag="xn", name="xn")
                    wsq = None
                    if j >= 6:
                        wsq = scr.tile([128, 4, BLK], FP32R, tag="wsq", bufs=1, name="wsq")
                    for mc in range(4):
                        ps = ps_t.tile([128, BLK], FP32, tag="pt", bufs=3, name="psh")
                        for kc in range(4):
                            nc.tensor.matmul(
                                ps, wh_sb[:, (j - 1) * 4 + kc, mc * 128:(mc + 1) * 128],
                                x_cur[:, kc, :], start=(kc == 0), stop=(kc == 3))
                        nc.scalar.activation(out=x_next[:, mc, :], in_=ps, func=AF.Relu)
                        if j >= 6:
                            nc.scalar.activation(
                                out=wsq[:, mc, :], in_=ps, func=AF.Square,
                                scale=sqs_sb[:, (j - 6) * 4 + mc:(j - 6) * 4 + mc + 1])
                        yield
                    if j >= 6:
                        w01 = scr.tile([128, BLK], FP32R, tag="ws2", bufs=2, name="w01")
                        nc.vector.tensor_add(w01, wsq[:, 0, :], wsq[:, 1, :])
                        w23 = scr.tile([128, BLK], FP32R, tag="ws2", bufs=2, name="w23")
                        nc.vector.tensor_add(w23, wsq[:, 2, :], wsq[:, 3, :])
                        gp = ps_share.tile([4, BLK], FP32, tag="gp", bufs=1, name="gp")
                        nc.tensor.matmul(gp, ones44_sb, w01, start=True, stop=False)
                        nc.tensor.matmul(gp, ones44_sb, w23, start=False, stop=True)
                        g = scr.tile([4, BLK], FP32, tag="gst", bufs=2, name=f"g{j}")
                        nc.scalar.copy(out=g, in_=gp)
                        g67[j - 6] = g
                    x_cur = x_next

                # ---- output layer ----
                yp = ps_share.tile([4, BLK], FP32, tag="gp", bufs=1, name="yp")
                for kc in range(4):
                    nc.tensor.matmul(
                        yp, wout_sb[:, kc, :], x_cur[:, kc, :],
                        start=(kc == 0), stop=(kc == 3))

                def finalize():
                    # c7^2 = S7 + eps*S6 ; out = [tanh(y/c7), 255*y/c7]
                    # rg = rsqrt(c7^2) via bit-trick seed + 2 Newton steps
                    # (keeps ACT in one table set: no Sqrt, no Reciprocal)
                    g4 = scr.tile([4, BLK], FP32, tag="g4", bufs=1, name="g4")
                    nc.vector.scalar_tensor_tensor(
                        out=g4, in0=g67[0], scalar=EPS, in1=g67[1],
                        op0=ALU.mult, op1=ALU.add)
                    g4h = scr.tile([4, BLK], FP32, tag="g4h", bufs=1, name="g4h")
                    nc.vector.tensor_scalar_mul(out=g4h, in0=g4, scalar1=0.5)
                    rg = scr.tile([4, BLK], INT32, tag="rg", bufs=1, name="rg")
                    nc.vector.tensor_scalar(
                        out=rg, in0=g4.bitcast(INT32), scalar1=1, scalar2=None,
                        op0=ALU.logical_shift_right)
                    nc.vector.tensor_scalar(
                        out=rg, in0=rg, scalar1=-1, scalar2=0x5F3759DF,
                        op0=ALU.mult, op1=ALU.add)
                    y0 = rg.bitcast(FP32)
                    nt = scr.tile([4, BLK], FP32, tag="nt", bufs=1, name="nt")
                    for _it in range(2):
                        nc.vector.tensor_tensor(out=nt, in0=y0, in1=y0, op=ALU.mult)
                        nc.vector.tensor_tensor(out=nt, in0=nt, in1=g4h, op=ALU.mult)
                        nc.vector.tensor_scalar(
                            out=nt, in0=nt, scalar1=-1.0, scalar2=1.5,
                            op0=ALU.mult, op1=ALU.add)
                        nc.vector.tensor_tensor(out=y0, in0=y0, in1=nt, op=ALU.mult)
                    yv = scr.tile([4, BLK], FP32, tag="yv", bufs=1, name="yv")
                    nc.vector.scalar_tensor_tensor(
                        out=yv, in0=yp, scalar=255.0, in1=y0,
                        op0=ALU.mult, op1=ALU.mult)
                    nc.scalar.activation(
                        out=yv[0:1, :], in_=yv[0:1, :], func=AF.Tanh, scale=1.0 / 255.0)
                    nc.sync.dma_start(out=outT_d[:, b * BLK:(b + 1) * BLK], in_=yv)

                yield finalize

            # ---- block loop, software-pipelined ----
            for s in steps0:
                s()                                    # prologue: P(0) inline
            finalize_prev = None
            for b in range(n_blocks):
                pre_cur = pre_next
                if b + 1 < n_blocks:
                    pre_next, steps_next = make_pre(b + 1)
                else:
                    steps_next = []
                step_i = 0
                if finalize_prev is not None:
                    finalize_prev()
                gen = emit_M(b, pre_cur)
                for out in gen:
                    if out is not None:
                        finalize_prev = out
                        break
                    if step_i < len(steps_next):
                        steps_next[step_i]()
                        step_i += 1
                while step_i < len(steps_next):        # drain leftovers
                    steps_next[step_i]()
                    step_i += 1
            finalize_prev()

    nc.compile()
    return nc


def kernel(**inputs):
    if _general_case_needed(inputs):
        return _numpy_fallback(inputs)

    from concourse.bass_utils import run_bass_kernel_spmd

    pre = _precompute(inputs)
    inp = np.ascontiguousarray(np.asarray(inputs["input"], np.float32))

    if "nc" not in _NC_CACHE:
        _NC_CACHE["nc"] = _build_bass()
    nc = _NC_CACHE["nc"]

    in_maps = [
        {
            "inp": np.ascontiguousarray(inp[c * S_CORE:(c + 1) * S_CORE]),
            "w0p": pre["w0p"], "whp": pre["whp"], "sqs": pre["sqs"],
            "gq": pre["gq"], "sel7": pre["sel7"], "ident": pre["ident"],
            "ones44": pre["ones44"], "woutp": pre["woutp"],
        }
        for c in range(N_CORES)
    ]

    res = run_bass_kernel_spmd(
        nc, in_maps, core_ids=list(range(N_CORES)),
        trace=bool(int(os.environ.get("KERNEL_TRACE", "0"))),
    )
    kernel.last_results = res
    outs = [res.results[c]["outT"] for c in range(N_CORES)]
    return np.ascontiguousarray(
        np.concatenate([o.T for o in outs], axis=0).astype(np.float32)
    )


# revision 13
# speedup vs baseline: 1.0354x; 1.0354x over previous
"""Trainium2 Bass kernel for nn_Decoder (latent-grid decoder MLP).

Contract: kernel(**inputs) takes the FULL unsharded inputs (as produced by
setup_inputs()) and returns the FULL [65536, 4] float32 output. Internally the
65536 points are sharded across 8 NeuronCores (pure data parallel); the small
weights are replicated.

Algorithm (mathematically equivalent to the reference):
  - G=2 trilinear interp of a per-sample 2x2x2 grid always lands in cell
    (0,0,0), so lat_i = sum_m w_m(xyz) * (lat @ A_m).
  - Expressed in the MONOMIAL basis: lat_i @ W0_top = lat@D_0 + sum_{S}
    mono_S(f) * (lat @ D_S) where D_S are alternating sums of the corner
    matrices folded with W0 (host-side).  u = [lat, fx*lat, ..., fxfyfz*lat,
    sin, cos] (2304 dims), h0 = u @ M0.
  - LayerNorm mean-subtraction and gamma fold into the weights; the per-sample
    rstd is deferred via LN's positive scale invariance.  Only the last two
    layers' sum-of-squares are needed: c7^2 = S7 + eps*S6, out = y/c7.
    S_j is accumulated by the PE with a per-partition-scaled Square on ACT
    (scale 1/(sqrt(512)*g)) and an all-ones [128,4] stationary.
  - Per-block schedule is software-pipelined: the whole preamble of block b+1
    (input transposes, trilinear monomials, Fourier range reduction + sin/cos,
    broadcast matmuls and u-chunk products) is emitted interleaved with block
    b's MLP layers, so the PE sees a dense stream of 512-row fp32r matmuls.
Activations live in [feature, sample] layout; matmuls run as fp32r (full PE
rate at N=512).
"""

import os
import numpy as np

N_CORES = 8
N_TOTAL = 65536
S_CORE = N_TOTAL // N_CORES          # 8192 samples per core
BLK = 512                            # samples per block
N_BLOCKS = S_CORE // BLK             # 16
EPS = 1e-5
N_LAYERS = 8                         # LN+relu layers (layer0 + 7 hidden)


def _precompute(inputs):
    """Host-side weight folding. Returns dict of constant arrays (fp32)."""
    convT_w = np.asarray(inputs["convT_w"], np.float32)
    W0 = np.asarray(inputs["W0"], np.float32)
    Wh = np.asarray(inputs["Wh"], np.float32)
    ln_g = np.asarray(inputs["ln_g"], np.float32)
    gauss = np.asarray(inputs["gauss"], np.float32)
    W_out = np.asarray(inputs["W_out"], np.float32)

    # corner-folded first-layer weights: B[d] = A_d @ W0_top, [2,2,2,256,512]
    A = convT_w.transpose(2, 3, 4, 0, 1).reshape(8, 256, 512)
    B = (A @ W0[:512]).reshape(2, 2, 2, 256, 512)
    # monomial basis D_S = sum_{d subset S} (-1)^{|S|-|d|} B_d
    D = np.empty((8, 256, 512), np.float32)
    D[0] = B[0, 0, 0]
    D[1] = B[1, 0, 0] - B[0, 0, 0]                                   # fx
    D[2] = B[0, 1, 0] - B[0, 0, 0]                                   # fy
    D[3] = B[0, 0, 1] - B[0, 0, 0]                                   # fz
    D[4] = B[1, 1, 0] - B[1, 0, 0] - B[0, 1, 0] + B[0, 0, 0]         # fx fy
    D[5] = B[1, 0, 1] - B[1, 0, 0] - B[0, 0, 1] + B[0, 0, 0]         # fx fz
    D[6] = B[0, 1, 1] - B[0, 1, 0] - B[0, 0, 1] + B[0, 0, 0]         # fy fz
    D[7] = (B[1, 1, 1] - B[1, 1, 0] - B[1, 0, 1] - B[0, 1, 1]
            + B[1, 0, 0] + B[0, 1, 0] + B[0, 0, 1] - B[0, 0, 0])     # fx fy fz
    M0 = np.concatenate([D.reshape(2048, 512), W0[512:640], W0[640:768]], axis=0)

    def center_scale(W, g):
        Wc = W - W.mean(axis=1, keepdims=True)
        return np.ascontiguousarray(Wc * g[None, :], np.float32)

    W_eff = [center_scale(M0, ln_g[0])] + [
        center_scale(Wh[l], ln_g[l + 1]) for l in range(7)
    ]

    # pack each layer's weights as [128, n_kchunks, 512]
    def pack(W):
        K = W.shape[0]
        kc = K // 128
        return W.reshape(kc, 128, 512).transpose(1, 0, 2).reshape(128, kc * 512)

    w0p = np.ascontiguousarray(pack(W_eff[0]))                       # [128, 18*512]
    whp = np.ascontiguousarray(
        np.concatenate([pack(W) for W in W_eff[1:]], axis=1))        # [128, 28*512]

    # per-partition ACT Square scales: col (j-6)*4+mc -> 1/(sqrt(512)*|g_j|)
    sqs = np.empty((128, 8), np.float32)
    for j in (6, 7):
        g = np.abs(ln_g[j]).astype(np.float32)
        for mc in range(4):
            sqs[:, (j - 6) * 4 + mc] = 1.0 / (np.sqrt(512.0) * g[mc * 128:(mc + 1) * 128])

    # gauss stationaries: [4, 2*128]; col block 0 = gauss.T (row 3 zero),
    # col block 1 = gauss.T with row 3 = 0.25 (cos phase shift, revolutions)
    gq = np.zeros((4, 256), np.float32)
    gq[0:3, 0:128] = gauss.T
    gq[0:3, 128:256] = gauss.T
    gq[3, 128:256] = 0.25

    # monomial broadcast selector, rows 32:39 (matmul tile_position row=32)
    sel7 = np.zeros((39, 7 * 128), np.float32)
    sel7[32:39] = np.kron(np.eye(7, dtype=np.float32), np.ones((1, 128), np.float32))

    return {
        "w0p": w0p,
        "whp": whp,
        "sqs": sqs,
        "gq": gq,
        "sel7": np.ascontiguousarray(sel7),
        "ident": np.eye(128, dtype=np.float32),
        "ones44": np.ones((128, 4), np.float32),
        "woutp": np.ascontiguousarray(
            W_out.reshape(4, 128, 4).transpose(1, 0, 2).reshape(128, 16)),
    }


def _general_case_needed(inputs):
    z = lambda a: bool(np.all(np.asarray(a) == 0))
    return not (
        z(inputs["convT_b"]) and z(inputs["b0"]) and z(inputs["bh"])
        and z(inputs["ln_b"]) and z(inputs["b_out"])
        and bool(np.all(np.abs(np.asarray(inputs["ln_g"])) > 1e-3))
    )


def _numpy_fallback(inputs):
    """Reference in numpy (slow; only for inputs outside the fast path)."""
    inp = np.asarray(inputs["input"], np.float32)
    convT_w = np.asarray(inputs["convT_w"], np.float32)
    convT_b = np.asarray(inputs["convT_b"], np.float32)
    gauss = np.asarray(inputs["gauss"], np.float32)
    W0 = np.asarray(inputs["W0"], np.float32)
    b0 = np.asarray(inputs["b0"], np.float32)
    Wh = np.asarray(inputs["Wh"], np.float32)
    bh = np.asarray(inputs["bh"], np.float32)
    ln_g = np.asarray(inputs["ln_g"], np.float32)
    ln_b = np.asarray(inputs["ln_b"], np.float32)
    W_out = np.asarray(inputs["W_out"], np.float32)
    b_out = np.asarray(inputs["b_out"], np.float32)
    xyz = inp[:, -3:]
    lat = inp[:, :-3]
    f = (xyz + 1.0) * 0.5
    frac = f - np.clip(f.astype(np.int32), 0, 0)
    A = convT_w.transpose(2, 3, 4, 0, 1)
    lat_i = np.zeros((inp.shape[0], 512), np.float32)
    wx = [1 - frac[:, 0], frac[:, 0]]
    wy = [1 - frac[:, 1], frac[:, 1]]
    wz = [1 - frac[:, 2], frac[:, 2]]
    for di in (0, 1):
        for dj in (0, 1):
            for dk in (0, 1):
                w = (wx[di] * wy[dj] * wz[dk]).astype(np.float32)
                lat_i += (lat @ A[di, dj, dk]) * w[:, None]
    lat_i += convT_b[None, :]
    ang = 2 * np.pi * (xyz @ gauss.T)
    x = np.concatenate([lat_i, np.sin(ang), np.cos(ang)], axis=1)

    def ln(t, g, b):
        mu = t.mean(-1, keepdims=True)
        var = ((t - mu) ** 2).mean(-1, keepdims=True)
        return (t - mu) / np.sqrt(var + EPS) * g + b

    x = np.maximum(ln(x @ W0 + b0, ln_g[0], ln_b[0]), 0)
    for l in range(7):
        x = np.maximum(ln(x @ Wh[l] + bh[l], ln_g[l + 1], ln_b[l + 1]), 0)
    y = x @ W_out + b_out
    return np.concatenate([np.tanh(y[:, :1]), y[:, 1:] * 255.0], axis=1).astype(np.float32)


_NC_CACHE = {}


def _build_bass(s_core=S_CORE):
    """Build the per-core Bass module (SPMD; same program on all 8 cores)."""
    import concourse.bass as bass
    import concourse.bacc as bacc
    import concourse.tile as tile
    from concourse import mybir

    FP32 = mybir.dt.float32
    FP32R = mybir.dt.float32r
    INT32 = mybir.dt.int32
    AF = mybir.ActivationFunctionType
    ALU = mybir.AluOpType
    TWO_PI = float(2.0 * np.pi)
    MAGIC = 12582912.0            # 1.5 * 2^23: fp32 add/sub rounds to integer
    n_blocks = s_core // BLK

    nc = bacc.Bacc("TRN2", target_bir_lowering=False, debug=False)

    inp_d = nc.dram_tensor("inp", [s_core, 259], FP32R, kind="ExternalInput").ap()
    w0p_d = nc.dram_tensor("w0p", [128, 18 * 512], FP32R, kind="ExternalInput").ap()
    whp_d = nc.dram_tensor("whp", [128, 28 * 512], FP32R, kind="ExternalInput").ap()
    sqs_d = nc.dram_tensor("sqs", [128, 8], FP32, kind="ExternalInput").ap()
    gq_d = nc.dram_tensor("gq", [4, 256], FP32R, kind="ExternalInput").ap()
    sel7_d = nc.dram_tensor("sel7", [39, 7 * 128], FP32R, kind="ExternalInput").ap()
    ident_d = nc.dram_tensor("ident", [128, 128], FP32R, kind="ExternalInput").ap()
    ones44_d = nc.dram_tensor("ones44", [128, 4], FP32R, kind="ExternalInput").ap()
    woutp_d = nc.dram_tensor("woutp", [128, 16], FP32R, kind="ExternalInput").ap()
    outT_d = nc.dram_tensor("outT", [4, s_core], FP32, kind="ExternalOutput").ap()

    def R(ap):
        return ap.bitcast(FP32R)

    with tile.TileContext(nc) as tc:
        with (
            tc.tile_pool(name="const", bufs=1) as constp,
            tc.tile_pool(name="weights", bufs=1) as weightp,
            tc.tile_pool(name="inblk", bufs=2) as inp_pool,
            tc.tile_pool(name="pre", bufs=2) as prep,
            tc.tile_pool(name="acts", bufs=2) as actp,
            tc.tile_pool(name="scratch", bufs=2) as scr,
            tc.tile_pool(name="ps_t", bufs=1, space="PSUM") as ps_t,
            tc.tile_pool(name="ps_share", bufs=1, space="PSUM") as ps_share,
            tc.tile_pool(name="ps_pre", bufs=1, space="PSUM") as ps_pre,
        ):
            inp_r = inp_d.rearrange("(b sc p) f -> b p sc f", sc=4, p=128)

            def make_pre(b):
                """Preamble for block b: returns (tiles dict, list of closures).

                Issues the input DMA immediately; everything else is deferred
                into steps that the caller pumps between M-phase matmul groups
                of block b-1.  Produces latT, movers (18 l0 moving chunks) in
                SBUF, all in [feature, sample] layout.
                """
                t = {}
                inb = inp_pool.tile([128, 4, 259], FP32R, tag="inb", name="inb")
                nc.sync.dma_start(out=inb, in_=inp_r[b])
                t["latT"] = prep.tile([128, 2, BLK], FP32R, tag="latT", name="latT")
                wxz = prep.tile([128, 4, 39], FP32R, tag="wxz", name="wxz")
                xyzq = prep.tile([4, BLK], FP32R, tag="xyzq", name="xyzq")
                w7T = prep.tile([39, BLK], FP32R, tag="w7T", name="w7T")
                ffs = prep.tile([128, BLK], FP32R, tag="ffs", name="ffs")
                ffc = prep.tile([128, BLK], FP32R, tag="ffc", name="ffc")
                uchs = [scr.tile([128, BLK], FP32R, tag="uch", bufs=14, name=f"uch{m}")
                        for m in range(14)]
                # l0 moving chunks in order matching M0 rows
                t["movers"] = [t["latT"][:, 0, :], t["latT"][:, 1, :]] + \
                    [uchs[i] for i in range(14)] + [ffs, ffc]
                steps = []

                def s_wxz():
                    # f = (xyz+1)/2 into monomial cols 32:35; products 35:39;
                    # raw xyz into 0:3; ones into col 3
                    nc.vector.tensor_scalar(
                        out=wxz[:, :, 32:35], in0=inb[:, :, 256:259],
                        scalar1=0.5, scalar2=0.5, op0=ALU.mult, op1=ALU.add)
                    nc.vector.tensor_tensor(
                        out=wxz[:, :, 35:36], in0=wxz[:, :, 32:33],
                        in1=wxz[:, :, 33:34], op=ALU.mult)              # fx fy
                    nc.vector.tensor_tensor(
                        out=wxz[:, :, 36:37], in0=wxz[:, :, 32:33],
                        in1=wxz[:, :, 34:35], op=ALU.mult)              # fx fz
                    nc.vector.tensor_tensor(
                        out=wxz[:, :, 37:38], in0=wxz[:, :, 33:34],
                        in1=wxz[:, :, 34:35], op=ALU.mult)              # fy fz
                    nc.vector.tensor_tensor(
                        out=wxz[:, :, 38:39], in0=wxz[:, :, 35:36],
                        in1=wxz[:, :, 34:35], op=ALU.mult)              # fx fy fz
                    nc.vector.tensor_copy(out=wxz[:, :, 0:3], in_=inb[:, :, 256:259])
                    nc.vector.tensor_scalar(
                        out=wxz[:, :, 3:4], in0=inb[:, :, 0:1],
                        scalar1=0.0, scalar2=1.0, op0=ALU.mult, op1=ALU.add)
                steps.append(s_wxz)

                # per-sc: 2 lat transposes + 1 combined xyz/monomial transpose
                def mk_lat_tp(sc, fc):
                    def s():
                        tp = ps_pre.tile([128, 128], FP32R, tag="tp", bufs=1, name="tp")
                        nc.tensor.transpose(
                            tp, inb[:, sc, fc * 128:(fc + 1) * 128], ident_sb)
                        nc.vector.tensor_copy(
                            t["latT"][:, fc, sc * 128:(sc + 1) * 128], tp)
                    return s

                def mk_wxz_tp(sc):
                    def s():
                        tp = ps_pre.tile([39, 128], FP32R, tag="tp", bufs=1, name="tpw")
                        nc.tensor.transpose(tp, wxz[:, sc, :], ident_sb)
                        nc.vector.tensor_copy(
                            xyzq[:, sc * 128:(sc + 1) * 128], tp[0:4, :])
                        nc.vector.tensor_copy(
                            w7T[32:39, sc * 128:(sc + 1) * 128], tp[32:39, :])
                    return s

                for sc in range(4):
                    steps.append(mk_lat_tp(sc, 0))
                    steps.append(mk_lat_tp(sc, 1))
                    steps.append(mk_wxz_tp(sc))

                # fourier: ang matmul + range reduce + sin (and cos phase)
                def mk_ang(col, zname, fout):
                    def s():
                        k = 3 if col == 0 else 4
                        angp = ps_pre.tile([128, BLK], FP32, tag="ang", bufs=1, name="angp")
                        nc.tensor.matmul(
                            angp, gq_sb[0:k, col, :], xyzq[0:k, :],
                            start=True, stop=True)
                        zr = scr.tile([128, BLK], FP32, tag="zr", bufs=1, name=zname + "r")
                        nc.vector.tensor_scalar(
                            out=zr, in0=angp, scalar1=MAGIC, scalar2=MAGIC,
                            op0=ALU.add, op1=ALU.subtract)
                        zz = scr.tile([128, BLK], FP32, tag="zz", bufs=1, name=zname)
                        nc.vector.tensor_sub(zz, angp, zr)
                        nc.scalar.activation(out=fout, in_=zz, func=AF.Sin, scale=TWO_PI)
                    return s

                steps.append(mk_ang(0, "zs", ffs))
                steps.append(mk_ang(1, "zc", ffc))

                # broadcast + u-chunk products (consumed by l0 of block b)
                def mk_bc(m):
                    def s():
                        bcp = ps_share.tile([128, BLK], FP32, tag="bc", bufs=2, name="bcp")
                        nc.tensor.matmul(
                            bcp, sel7_sb[32:39, m, :], w7T[32:39, :],
                            start=True, stop=True, tile_position=(32, 0))
                        nc.vector.tensor_tensor(
                            out=uchs[2 * m], in0=t["latT"][:, 0, :], in1=bcp,
                            op=ALU.mult)
                        nc.vector.tensor_tensor(
                            out=uchs[2 * m + 1], in0=t["latT"][:, 1, :], in1=bcp,
                            op=ALU.mult)
                    return s

                for m in range(7):
                    steps.append(mk_bc(m))
                return t, steps

            pre_next, steps0 = make_pre(0)

            # ---- constants / weights (loaded once, resident) ----
            # (first block's input DMA is issued below, before these bulk
            # weight transfers, so the prologue transposes can start early)
            w0_sb = weightp.tile([128, 18, 512], FP32R)
            w0r = w0p_d.rearrange("p (c f) -> p c f", c=18)
            for ch in range(3):
                nc.sync.dma_start(
                    out=w0_sb[:, ch * 6:(ch + 1) * 6, :], in_=w0r[:, ch * 6:(ch + 1) * 6, :])
            wh_sb = weightp.tile([128, 28, 512], FP32R)
            whr = whp_d.rearrange("p (c f) -> p c f", c=28)
            for ch in range(4):
                nc.sync.dma_start(
                    out=wh_sb[:, ch * 7:(ch + 1) * 7, :], in_=whr[:, ch * 7:(ch + 1) * 7, :])
            sqs_sb = constp.tile([128, 8], FP32)
            nc.sync.dma_start(out=sqs_sb, in_=sqs_d)
            gq_sb = constp.tile([4, 2, 128], FP32R)
            nc.sync.dma_start(out=gq_sb, in_=gq_d.rearrange("p (c f) -> p c f", c=2))
            sel7_sb = constp.tile([39, 7, 128], FP32R)
            nc.sync.dma_start(out=sel7_sb, in_=sel7_d.rearrange("p (m f) -> p m f", m=7))
            ident_sb = constp.tile([128, 128], FP32R)
            nc.sync.dma_start(out=ident_sb, in_=ident_d)
            ones44_sb = constp.tile([128, 4], FP32R)
            nc.sync.dma_start(out=ones44_sb, in_=ones44_d)
            wout_sb = weightp.tile([128, 4, 4], FP32R)
            nc.sync.dma_start(out=wout_sb, in_=woutp_d.rearrange("p (c f) -> p c f", c=4))

            def emit_M(b, pre):
                """MLP phase for block b; returns a finalize closure."""
                movers = pre["movers"]

                def finalize_stub():
                    pass

                # ---- layer 0: mc-major, 18 accumulating matmuls per chunk ----
                x_cur = actp.tile([128, 4, BLK], FP32R, tag="xn", name="x0")
                for mc in range(4):
                    ps = ps_t.tile([128, BLK], FP32, tag="pt", bufs=3, name="ps0")
                    for kc in range(18):
                        nc.tensor.matmul(
                            ps, w0_sb[:, kc, mc * 128:(mc + 1) * 128], movers[kc],
                            start=(kc == 0), stop=(kc == 17))
                    nc.scalar.activation(out=x_cur[:, mc, :], in_=ps, func=AF.Relu)
                    yield
                # ---- hidden layers ----
                g67 = [None, None]
                stats_pending = []
                for j in range(1, N_LAYERS):
                    x_next = actp.tile([128, 4, BLK], FP32R, tag="xn", name="xn")
                    wsq = None
                    if j >= 6:
                        wsq = scr.tile([128, 4, BLK], FP32R, tag="wsq", bufs=1, name="wsq")
                    for mc in range(4):
                        ps = ps_t.tile([128, BLK], FP32, tag="pt", bufs=3, name="psh")
                        for kc in range(4):
                            nc.tensor.matmul(
                                ps, wh_sb[:, (j - 1) * 4 + kc, mc * 128:(mc + 1) * 128],
                                x_cur[:, kc, :], start=(kc == 0), stop=(kc == 3))
                        nc.scalar.activation(out=x_next[:, mc, :], in_=ps, func=AF.Relu)
                        if j >= 6:
                            nc.scalar.activation(
                                out=wsq[:, mc, :], in_=ps, func=AF.Square,
                                scale=sqs_sb[:, (j - 6) * 4 + mc:(j - 6) * 4 + mc + 1])
                        yield
                    if j >= 6:
                        w01 = scr.tile([128, BLK], FP32R, tag="ws2", bufs=2, name="w01")
                        nc.vector.tensor_add(w01, wsq[:, 0, :], wsq[:, 1, :])
                        w23 = scr.tile([128, BLK], FP32R, tag="ws2", bufs=2, name="w23")
                        nc.vector.tensor_add(w23, wsq[:, 2, :], wsq[:, 3, :])
                        nc.vector.tensor_add(w01, w01, w23)
                        stats_pending.append((j, w01))
                    x_cur = x_next

                # ---- output layer (before stats matmuls: PE does not wait
                # on the DVE square-sums), then the two stats matmuls ----
                yp = ps_share.tile([4, BLK], FP32, tag="bc", bufs=2, name="yp")
                for kc in range(4):
                    nc.tensor.matmul(
                        yp, wout_sb[:, kc, :], x_cur[:, kc, :],
                        start=(kc == 0), stop=(kc == 3))
                for jj, w03 in stats_pending:
                    gp = ps_share.tile([4, BLK], FP32, tag="gp", bufs=1, name="gp")
                    nc.tensor.matmul(gp, ones44_sb, w03, start=True, stop=True)
                    g = scr.tile([4, BLK], FP32, tag="gst", bufs=2, name=f"g{jj}")
                    nc.scalar.copy(out=g, in_=gp)
                    g67[jj - 6] = g

                def finalize():
                    # c7^2 = S7 + eps*S6 ; out = [tanh(y/c7), 255*y/c7]
                    # rg = rsqrt(c7^2) via bit-trick seed + 2 Newton steps
                    # (keeps ACT in one table set: no Sqrt, no Reciprocal)
                    g4 = scr.tile([4, BLK], FP32, tag="g4", bufs=1, name="g4")
                    nc.vector.scalar_tensor_tensor(
                        out=g4, in0=g67[0], scalar=EPS, in1=g67[1],
                        op0=ALU.mult, op1=ALU.add)
                    g4h = scr.tile([4, BLK], FP32, tag="g4h", bufs=1, name="g4h")
                    nc.vector.tensor_scalar_mul(out=g4h, in0=g4, scalar1=0.5)
                    rg = scr.tile([4, BLK], INT32, tag="rg", bufs=1, name="rg")
                    nc.vector.tensor_scalar(
                        out=rg, in0=g4.bitcast(INT32), scalar1=1, scalar2=None,
                        op0=ALU.logical_shift_right)
                    nc.vector.tensor_scalar(
                        out=rg, in0=rg, scalar1=-1, scalar2=0x5F3759DF,
                        op0=ALU.mult, op1=ALU.add)
                    y0 = rg.bitcast(FP32)
                    nt = scr.tile([4, BLK], FP32, tag="nt", bufs=1, name="nt")
                    for _it in range(2):
                        nc.vector.tensor_tensor(out=nt, in0=y0, in1=y0, op=ALU.mult)
                        nc.vector.tensor_tensor(out=nt, in0=nt, in1=g4h, op=ALU.mult)
                        nc.vector.tensor_scalar(
                            out=nt, in0=nt, scalar1=-1.0, scalar2=1.5,
                            op0=ALU.mult, op1=ALU.add)
                        nc.vector.tensor_tensor(out=y0, in0=y0, in1=nt, op=ALU.mult)
                    yv = scr.tile([4, BLK], FP32, tag="yv", bufs=1, name="yv")
                    nc.vector.scalar_tensor_tensor(
                        out=yv, in0=yp, scalar=255.0, in1=y0,
                        op0=ALU.mult, op1=ALU.mult)
                    # tanh(yv/255) on DVE (keeps ACT in one table set):
                    # clamp 3.5, tanh x ~ x(945+105x^2+x^4)/(945+420x^2+15x^4)
                    # scratch rows reuse finalize tiles that are dead by now
                    tr = yv[0:1, :]
                    nc.vector.tensor_scalar(
                        out=tr, in0=tr, scalar1=1.0 / 255.0, scalar2=3.5,
                        op0=ALU.mult, op1=ALU.min)
                    nc.vector.tensor_scalar_max(out=tr, in0=tr, scalar1=-3.5)
                    x2 = g4[0:1, :]
                    nc.vector.tensor_tensor(out=x2, in0=tr, in1=tr, op=ALU.mult)
                    x4 = g4h[0:1, :]
                    nc.vector.tensor_tensor(out=x4, in0=x2, in1=x2, op=ALU.mult)
                    nm = nt[0:1, :]
                    nc.vector.tensor_scalar(
                        out=nm, in0=x2, scalar1=105.0, scalar2=945.0,
                        op0=ALU.mult, op1=ALU.add)
                    nc.vector.tensor_add(nm, nm, x4)
                    nc.vector.tensor_tensor(out=nm, in0=nm, in1=tr, op=ALU.mult)
                    dn = scr.tile([4, BLK], FP32, tag="tnh", bufs=1, name="tnh")[0:1, :]
                    nc.vector.tensor_scalar(
                        out=dn, in0=x2, scalar1=420.0, scalar2=945.0,
                        op0=ALU.mult, op1=ALU.add)
                    nc.vector.scalar_tensor_tensor(
                        out=dn, in0=x4, scalar=15.0, in1=dn,
                        op0=ALU.mult, op1=ALU.add)
                    rr = rg[0:1, :]
                    nc.vector.tensor_scalar(
                        out=rr, in0=dn.bitcast(INT32), scalar1=-1, scalar2=0x7EF311C3,
                        op0=ALU.mult, op1=ALU.add)
                    rf = rr.bitcast(FP32)
                    nt2 = x2
                    for _it in range(2):
                        nc.vector.tensor_tensor(out=nt2, in0=dn, in1=rf, op=ALU.mult)
                        nc.vector.tensor_scalar(
                            out=nt2, in0=nt2, scalar1=-1.0, scalar2=2.0,
                            op0=ALU.mult, op1=ALU.add)
                        nc.vector.tensor_tensor(out=rf, in0=rf, in1=nt2, op=ALU.mult)
                    nc.vector.tensor_tensor(out=tr, in0=nm, in1=rf, op=ALU.mult)
                    nc.sync.dma_start(out=outT_d[:, b * BLK:(b + 1) * BLK], in_=yv)

                yield finalize

            # ---- block loop, software-pipelined ----
            for s in steps0:
                s()                                    # prologue: P(0) inline
            finalize_prev = None
            for b in range(n_blocks):
                pre_cur = pre_next
                if b + 1 < n_blocks:
                    pre_next, steps_next = make_pre(b + 1)
                else:
                    steps_next = []
                step_i = 0
                if finalize_prev is not None:
                    finalize_prev()
                gen = emit_M(b, pre_cur)
                for out in gen:
                    if out is not None:
                        finalize_prev = out
                        break
                    for _ in range(2):
                        if step_i < len(steps_next):
                            steps_next[step_i]()
                            step_i += 1
                while step_i < len(steps_next):        # drain leftovers
                    steps_next[step_i]()
                    step_i += 1
            finalize_prev()

    nc.compile()
    return nc


def kernel(**inputs):
    if _general_case_needed(inputs):
        return _numpy_fallback(inputs)

    from concourse.bass_utils import run_bass_kernel_spmd

    pre = _precompute(inputs)
    inp = np.ascontiguousarray(np.asarray(inputs["input"], np.float32))

    if "nc" not in _NC_CACHE:
        _NC_CACHE["nc"] = _build_bass()
    nc = _NC_CACHE["nc"]

    in_maps = [
        {
            "inp": np.ascontiguousarray(inp[c * S_CORE:(c + 1) * S_CORE]),
            "w0p": pre["w0p"], "whp": pre["whp"], "sqs": pre["sqs"],
            "gq": pre["gq"], "sel7": pre["sel7"], "ident": pre["ident"],
            "ones44": pre["ones44"], "woutp": pre["woutp"],
        }
        for c in range(N_CORES)
    ]

    res = run_bass_kernel_spmd(
        nc, in_maps, core_ids=list(range(N_CORES)),
        trace=bool(int(os.environ.get("KERNEL_TRACE", "0"))),
    )
    kernel.last_results = res
    outs = [res.results[c]["outT"] for c in range(N_CORES)]
    return np.ascontiguousarray(
        np.concatenate([o.T for o in outs], axis=0).astype(np.float32)
    )
